# revision 17
# baseline (speedup 1.0000x reference)
"""Trainium2 Bass kernel: 12-head attention (B=2, N=2048, C=768) on 8 NeuronCores.

Sharding: core c -> batch b = c // 4, head-group g = c % 4 (heads 3g..3g+2).

Key optimizations over the naive head-sharded layout:

- Mask compaction: the key mask is host-visible, so tokens of each batch are
  permuted so kept keys (~1002/1034 of 2048) come first; K/V/attention only
  process ceil(max_kept/128)*128 keys instead of 2048 (QK, exp, PV all shrink
  ~45%). Queries still cover all 2048 tokens; the host un-permutes at the end.
  Keys beyond the kept count are real (masked) tokens whose V rows and
  softmax-denominator column are zeroed via the mf vector, so they contribute
  exactly 0 to numerator and denominator (matching the reference).

- Head packing on the PE array: wq/wk are laid out [q0|q1] / [k0|k1] so head 0
  lives on SBUF partitions 0-63 and head 1 on 64-127. QK^T has contraction 64,
  so head-0/head-1 matmuls occupy disjoint row-halves of the 128x128 array
  (tile_position auto-derived from base partitions) and run concurrently, and
  their LDWEIGHTS overlap the other head's in-flight matmuls. Heads 0/1 are
  processed chunk-interleaved in one "pair unit" (two PSUM o-accumulators);
  head 2 uses duplicated columns [q2|q2] / [k2|k2] and alternates halves.

- The attention phase is scalar-engine(exp)-bound, so everything else hides
  under it: the kst[:,1]/qs2/V projections run as fill-in work inserted
  between attention chunks; attention starts as soon as the k0|k1 and
  half-0 q0|q1 projections land instead of after the full QKV phase. PV
  matmuls for chunk c are emitted after chunk c+1's QK so the FIFO tensor
  queue never head-of-line blocks on the exp.

- Softmax denominator via an extra all-ones column appended to V (one PV
  matmul yields values + row sums). Normalization never touches the scalar
  engine (DVE copy + gpsimd row-copy/broadcast + DVE reciprocal/multiply),
  and the PSUM accumulator is released after a single copy so the next
  unit's PV can start immediately (keeps the PE HAM-warm).

- Fully-useful 8-way AllToAll per query half: each half's 1024 tokens split
  into 8 blocks of 128; receiver j gets all 768 channels of token-block j for
  BOTH batches, so the projection runs on 256 columns (128 tokens x 2
  batches) per half with zero wasted wire bytes or flops. A2A#0 and proj#0
  hide under the second half's attention; during the exposed A2A#1 the PE
  runs proj#0 plus warm-up filler matmuls so the final projection executes
  at full clock. proj DMAs are sequenced before the next collective because
  DMAs issued after a collective barrier-wait on it.
"""

import numpy as np
import ml_dtypes

B, N, C = 2, 2048, 768
H, HD = 12, 64
HPG = 3            # heads per core
GPB = 4            # cores (head-groups) per batch
NCORES = 8
SCALE = float(HD) ** -0.5
DCH = C // 128     # 6 contraction chunks
NH = N // 2        # 1024 query tokens per half

bf = ml_dtypes.bfloat16

_cache = {}


def _build(nkch):
    import concourse.mybir as mybir
    import concourse.tile as tile
    from concourse import bacc

    fp32 = mybir.dt.float32
    bfl = mybir.dt.bfloat16
    EXP = mybir.ActivationFunctionType.Exp
    MULT = mybir.AluOpType.mult

    NK = nkch * 128  # padded key count

    nc = bacc.Bacc(None, num_devices=NCORES)
    xT = nc.declare_dram_parameter("xT", [C, N], bfl, isOutput=False)
    wq = nc.declare_dram_parameter("wq", [C, 256], bfl, isOutput=False)
    wk = nc.declare_dram_parameter("wk", [C, 256], bfl, isOutput=False)
    wv = nc.declare_dram_parameter("wv", [C, HPG * HD], bfl, isOutput=False)
    wp = nc.declare_dram_parameter("wp", [C, C], bfl, isOutput=False)
    bp = nc.declare_dram_parameter("bp", [128, DCH], fp32, isOutput=False)
    mf = nc.declare_dram_parameter("mf", [128, nkch], fp32, isOutput=False)
    out = nc.declare_dram_parameter("out", [C, 512], fp32, isOutput=True)

    with tile.TileContext(nc) as tc:
        with (
            tc.tile_pool(name="const", bufs=1) as cpool,
            tc.tile_pool(name="work", bufs=1) as wpool,
            tc.tile_pool(name="pp", bufs=2) as ppool,
        ):
            # ---------------- input loads (order = need order) ----------------
            mf_sb = cpool.tile([128, nkch], fp32, tag="mf")
            nc.sync.dma_start(mf_sb[:], mf[:])
            wk_sb = cpool.tile([128, DCH, 256], bfl, tag="wk")
            nc.sync.dma_start(wk_sb[:], wk.rearrange("(o p) c -> p o c", p=128))
            wq_sb = cpool.tile([128, DCH, 256], bfl, tag="wq")
            nc.sync.dma_start(wq_sb[:], wq.rearrange("(o p) c -> p o c", p=128))
            xT_sb = cpool.tile([128, DCH, N], bfl, tag="xT")
            xT_r = xT.rearrange("(o p) t -> p o t", p=128)
            nc.sync.dma_start(xT_sb[:, :, 0:512], xT_r[:, :, 0:512])
            wv_sb = cpool.tile([128, DCH, HPG * HD], bfl, tag="wv")
            nc.sync.dma_start(wv_sb[:], wv.rearrange("(o p) c -> p o c", p=128))
            for tq in range(1, 4):
                nc.sync.dma_start(
                    xT_sb[:, :, tq * 512 : (tq + 1) * 512],
                    xT_r[:, :, tq * 512 : (tq + 1) * 512],
                )
            wp_sb = cpool.tile([128, DCH, C], bfl, tag="wp")
            nc.sync.dma_start(wp_sb[:], wp.rearrange("(o p) c -> p o c", p=128))
            bp_sb = cpool.tile([128, DCH], fp32, tag="bp")
            nc.sync.dma_start(bp_sb[:], bp[:])

            # preload the exp table set while DMAs run
            warm = cpool.tile([1, 8], fp32, tag="warm")
            nc.vector.memset(warm[:], 0.0)
            nc.scalar.activation(warm[:], warm[:], EXP)

            qs = wpool.tile([128, N], bfl, tag="qs")      # [q0 | q1] channel-major
            qs2 = wpool.tile([128, N], bfl, tag="qs2")    # [q2 | q2]
            kst = wpool.tile([128, 2, NK], bfl, tag="kst")  # [:,0]=[k0|k1] [:,1]=[k2|k2]
            V3 = wpool.tile([128, nkch, HPG, HD + 1], bfl, tag="V3")

            # PSUM: tag "s" 2 slots x 2 banks (QK scores + all projection /
            # fill-in tiles), tag "o" 2 slots x 2 banks (live PV accumulators).
            sps_cm = tc.tile_pool(name="sps", bufs=2, space="PSUM")
            sps = sps_cm.__enter__()
            ops_cm = tc.tile_pool(name="ops", bufs=2, space="PSUM")
            ops = ops_cm.__enter__()

            def qk_pass(which, m, tq):
                """Q or K projection Mtile m over token quarter tq (512)."""
                lo = tq * 512
                w_sb = wq_sb if which == "q" else wk_sb
                wid = min(512, (NK - lo) if which == "k" else 512)
                if wid <= 0:
                    return
                t = sps.tile([128, NH], fp32, tag="s", name="qk_t")[:, :wid]
                for kk in range(DCH):
                    nc.tensor.matmul(
                        t[:],
                        lhsT=w_sb[:, kk, m * 128 : (m + 1) * 128],
                        rhs=xT_sb[:, kk, lo : lo + wid],
                        start=(kk == 0),
                        stop=(kk == DCH - 1),
                    )
                dst = (qs if m == 0 else qs2) if which == "q" else None
                if which == "q":
                    nc.vector.tensor_copy(dst[:, lo : lo + wid], t[:])
                else:
                    nc.vector.tensor_copy(kst[:, m, lo : lo + wid], t[:])

            def v_pass(c):
                """V projection for key chunk c -> V3 (values * mf, ones col)."""
                v_t = sps.tile([128, NH], fp32, tag="s", name="v_t")[:, : HPG * HD]
                for kk in range(DCH):
                    nc.tensor.matmul(
                        v_t[:],
                        lhsT=xT_sb[:, kk, c * 128 : (c + 1) * 128],
                        rhs=wv_sb[:, kk, :],
                        start=(kk == 0),
                        stop=(kk == DCH - 1),
                    )
                nc.vector.tensor_scalar_mul(
                    V3[:, c, :, 0:HD],
                    v_t[:].rearrange("p (h d) -> p h d", h=HPG),
                    mf_sb[:, c : c + 1],
                )
                nc.vector.tensor_copy(
                    V3[:, c, :, HD], mf_sb[:, c : c + 1].to_broadcast((128, HPG))
                )

            OnA = [wpool.tile([128, NH], bfl, tag=f"OnA{q}", name=f"OnA{q}") for q in range(2)]
            OnB = [wpool.tile([64, NH], bfl, tag=f"OnB{q}", name=f"OnB{q}") for q in range(2)]

            def normalize(h, qh, o_t):
                """osb <- o (frees PSUM fast); rb = 1/rowsum bcast; OnX = o*rb."""
                osb = wpool.tile([HD + 1, NH], fp32, tag="osb", bufs=2, name="osb")
                nc.vector.tensor_copy(osb[:], o_t[:])
                sums = wpool.tile([1, NH], fp32, tag="sums", bufs=2, name="sums")
                nc.vector.tensor_copy(sums[:], osb[HD : HD + 1, :])
                rbb = wpool.tile([HD, NH], fp32, tag="rbb", bufs=2, name="rbb")
                nc.gpsimd.partition_broadcast(rbb[:], sums[:])
                rb = wpool.tile([HD, NH], fp32, tag="rb", bufs=2, name="rb")
                nc.vector.reciprocal_approx_fast(rb[:], rbb[:])
                dst = OnA[qh][h * 64 : (h + 1) * 64, :] if h < 2 else OnB[qh][:, :]
                nc.vector.tensor_tensor(dst, osb[0:HD, :], rb[:], MULT)

            def qk_mm(s_t, ksrc, qsrc, base, c, qh):
                for n2 in range(2):
                    nc.tensor.matmul(
                        s_t[:, n2 * 512 : (n2 + 1) * 512],
                        lhsT=ksrc[base : base + 64, c * 128 : (c + 1) * 128],
                        rhs=qsrc[
                            base : base + 64,
                            qh * NH + n2 * 512 : qh * NH + (n2 + 1) * 512,
                        ],
                        start=True,
                        stop=True,
                    )

            def pv_mm(o_t, p_t, c, h):
                for n2 in range(2):
                    nc.tensor.matmul(
                        o_t[:, n2 * 512 : (n2 + 1) * 512],
                        lhsT=V3[:, c, h, :],
                        rhs=p_t[:, c, n2 * 512 : (n2 + 1) * 512],
                        start=(c == 0),
                        stop=(c == nkch - 1),
                    )

            def pair_unit(qh, extras=()):
                """Heads 0+1, chunk-interleaved, query half qh. PV for chunk
                c-1 is emitted after chunk c's QK (FIFO queue stays unblocked).
                extras: thunks inserted one per chunk (fill-in projections)."""
                p_t = [
                    ppool.tile([128, nkch, NH], bfl, tag="p", name=f"pu{h}")
                    for h in range(2)
                ]
                o_t = [ops.tile([HD + 1, NH], fp32, tag="o", name=f"ou{h}") for h in range(2)]
                ex = list(extras)
                for c in range(nkch):
                    if c < len(ex) and ex[c] is not None:
                        ex[c]()
                    s_t = []
                    for h in range(2):
                        st = sps.tile([128, NH], fp32, tag="s", name=f"s{h}")
                        qk_mm(st, kst[:, 0], qs, 64 * h, c, qh)
                        s_t.append(st)
                    for h in range(2):
                        nc.scalar.activation(p_t[h][:, c, :], s_t[h][:], EXP, scale=SCALE)
                    if c > 0:
                        for h in range(2):
                            pv_mm(o_t[h], p_t[h], c - 1, h)
                for h in range(2):
                    pv_mm(o_t[h], p_t[h], nkch - 1, h)
                for h in range(2):
                    normalize(h, qh, o_t[h])

            def h2_unit(qh, extras=()):
                """Head 2 over query half qh; kst[:,1]/qs2 hold [k2|k2]/[q2|q2]
                so chunks alternate array row-halves."""
                p_t = ppool.tile([128, nkch, NH], bfl, tag="p", name="pu2")
                o_t = ops.tile([HD + 1, NH], fp32, tag="o", name="ou2")
                ex = list(extras)
                for c in range(nkch):
                    if c < len(ex) and ex[c] is not None:
                        ex[c]()
                    s_t = sps.tile([128, NH], fp32, tag="s", name="s2")
                    qk_mm(s_t, kst[:, 1], qs2, 64 * (c % 2), c, qh)
                    nc.scalar.activation(p_t[:, c, :], s_t[:], EXP, scale=SCALE)
                    if c > 0:
                        pv_mm(o_t, p_t, c - 1, 2)
                pv_mm(o_t, p_t, nkch - 1, 2)
                normalize(2, qh, o_t)

            ag_in = [
                nc.dram_tensor(f"ag_in{q}", [NCORES * HPG * HD, 128], bfl)
                for q in range(2)
            ]
            ag_out = [
                nc.dram_tensor(f"ag_out{q}", [NCORES * HPG * HD, 128], bfl)
                for q in range(2)
            ]

            def bounce(qh):
                agi_r = ag_in[qh].rearrange("(j p) t -> p j t", j=NCORES)
                nc.sync.dma_start(
                    agi_r[0:128, :, :],
                    OnA[qh][:, :].rearrange("p (j t) -> p j t", j=NCORES),
                )
                nc.sync.dma_start(
                    agi_r[128:192, :, :],
                    OnB[qh][:, :].rearrange("p (j t) -> p j t", j=NCORES),
                )

            def a2a(qh):
                nc.gpsimd.collective_compute(
                    "AllToAll",
                    mybir.AluOpType.bypass,
                    replica_groups=[[0, 1, 2, 3, 4, 5, 6, 7]],
                    ins=[ag_in[qh][:].opt()],
                    outs=[ag_out[qh][:].opt()],
                )

            out_r = out.rearrange("(o p) t -> p o t", p=128)

            def proj_dma(qh):
                at_sb = wpool.tile(
                    [128, 2, DCH, 128], bfl, tag="at", bufs=2, name="at_sb"
                )
                nc.sync.dma_start(
                    at_sb[:], ag_out[qh].rearrange("(b o p) t -> p b o t", p=128, b=2)
                )
                return at_sb

            def proj_mtile(qh, at_sb, m):
                y_ps = sps.tile([128, NH], fp32, tag="s", name="y_ps")[:, :256]
                for kk in range(DCH):
                    nc.tensor.matmul(
                        y_ps[:].rearrange("p (b t) -> p b t", b=2),
                        lhsT=wp_sb[:, kk, m * 128 : (m + 1) * 128],
                        rhs=at_sb[:, :, kk, :],
                        start=(kk == 0),
                        stop=(kk == DCH - 1),
                    )
                y_sb = wpool.tile([128, 256], fp32, tag="y", bufs=2, name="y_sb")
                nc.vector.tensor_scalar_add(y_sb[:], y_ps[:], bp_sb[:, m : m + 1])
                nc.sync.dma_start(out_r[:, m, qh * 256 : (qh + 1) * 256], y_sb[:])

            def proj_pass(qh, at_sb):
                for m in range(DCH):
                    proj_mtile(qh, at_sb, m)

            def warm_fill(n):
                """Junk matmuls that keep the PE HAM-warm while waiting."""
                for _ in range(n):
                    w_ps = sps.tile([128, NH], fp32, tag="s", name="w_ps")[:, :256]
                    nc.tensor.matmul(
                        w_ps[:], lhsT=wp_sb[:, 0, 0:128], rhs=wp_sb[:, 1, 0:256],
                        start=True, stop=True,
                    )

            # ---------------- schedule ----------------
            # Minimal bootstrap so the first exp lands as early as possible:
            # keys/queries for the first chunks only; everything else becomes
            # fill-in work inside the scalar-engine-bound attention units.
            nq = (NK + 511) // 512  # K token-quarters (3 for NK=1152)
            qk_pass("k", 0, 0)
            qk_pass("q", 0, 0)
            qk_pass("q", 0, 1)
            for c in range(min(4, nkch)):
                v_pass(c)

            vs = [lambda c=c: v_pass(c) for c in range(4, nkch)]
            # pair(0) fill-ins with deadlines: v(c) at slot <= c, k0 quarter q
            # by chunk 4q, q2 (qs2) before h2_unit(0).
            ex_p0 = [lambda: qk_pass("q", 1, 0), lambda: qk_pass("q", 1, 1)]
            ex_p0 += vs[:1]
            ex_p0 += [lambda: qk_pass("k", 0, 1)]
            ex_p0 += vs[1:2]
            ex_p0 += [lambda q=q: qk_pass("k", 0, q) for q in range(2, nq)]
            ex_p0 += vs[2:]
            ex_h0 = [lambda q=q: qk_pass("k", 1, q) for q in range(2, nq)]
            ex_h0 += [lambda: qk_pass("q", 0, 2), lambda: qk_pass("q", 0, 3)]
            ex_p1 = [lambda: qk_pass("q", 1, 2), lambda: qk_pass("q", 1, 3)]

            # ---- half 0 (tokens 0-1023)
            pair_unit(0, extras=ex_p0[:nkch])
            for f in ex_p0[nkch:]:
                f()
            qk_pass("k", 1, 0)
            qk_pass("k", 1, 1)
            h2_unit(0, extras=ex_h0[:nkch])
            for f in ex_h0[nkch:]:
                f()
            bounce(0)
            a2a(0)

            # ---- half 1 (tokens 1024-2047); A2A#0 hides under pair(1) and
            # proj#0 runs as fill-in inside h2(1) (emission order = PE order).
            pair_unit(1, extras=ex_p1)
            at0 = proj_dma(0)
            ex_h1 = [None] * max(0, nkch - DCH) + [
                lambda m=m: proj_mtile(0, at0, m) for m in range(DCH)
            ]
            h2_unit(1, extras=ex_h1[:nkch])
            for f in ex_h1[nkch:]:
                if f is not None:
                    f()
            bounce(1)
            a2a(1)
            warm_fill(60)
            at1 = proj_dma(1)
            proj_pass(1, at1)

            ops_cm.__exit__(None, None, None)
            sps_cm.__exit__(None, None, None)

    nc.finalize()
    return nc


def _prep(x, mask, w_qkv, w_proj, b_proj):
    """Host-side compaction: per-batch token permutation (kept keys first) and
    per-core input shards."""
    perms, counts = [], []
    for b in range(B):
        perm = np.argsort(1 - mask[b], kind="stable")
        perms.append(perm)
        counts.append(int(mask[b].sum()))
    nkch = max(1, int(np.ceil(max(counts) / 128)))
    NK = nkch * 128

    xTs = []
    mfs = []
    for b in range(B):
        xp = np.ascontiguousarray(x[b][perms[b]].T).astype(bf)
        xTs.append(xp)
        m = np.zeros(NK, dtype=np.float32)
        m[: counts[b]] = 1.0
        mfs.append(np.ascontiguousarray(m.reshape(nkch, 128).T))

    bp_t = np.ascontiguousarray(b_proj.astype(np.float32).reshape(DCH, 128).T)
    wp_t = w_proj.astype(bf)

    in_maps = []
    for c in range(NCORES):
        b, g = c // GPB, c % GPB
        heads = [3 * g, 3 * g + 1, 3 * g + 2]
        q_cols = [h * HD + d for h in (heads[0], heads[1], heads[2], heads[2]) for d in range(HD)]
        k_cols = [C + h * HD + d for h in (heads[0], heads[1], heads[2], heads[2]) for d in range(HD)]
        v_cols = [2 * C + h * HD + d for h in heads for d in range(HD)]
        in_maps.append(
            {
                "xT": xTs[b],
                "wq": np.ascontiguousarray(w_qkv[:, q_cols]).astype(bf),
                "wk": np.ascontiguousarray(w_qkv[:, k_cols]).astype(bf),
                "wv": np.ascontiguousarray(w_qkv[:, v_cols]).astype(bf),
                "wp": wp_t,
                "bp": bp_t,
                "mf": mfs[b],
            }
        )
    return in_maps, perms, nkch


def kernel(x, mask, w_qkv, w_proj, b_proj, _trace=False):
    from concourse.bass_utils import run_bass_kernel_spmd

    x = np.asarray(x, dtype=np.float32)
    mask = np.asarray(mask)
    w_qkv = np.asarray(w_qkv, dtype=np.float32)
    w_proj = np.asarray(w_proj, dtype=np.float32)
    b_proj = np.asarray(b_proj, dtype=np.float32)
    in_maps, perms, nkch = _prep(x, mask, w_qkv, w_proj, b_proj)
    if ("nc", nkch) not in _cache:
        _cache[("nc", nkch)] = _build(nkch)
    nc = _cache[("nc", nkch)]
    res = run_bass_kernel_spmd(nc, in_maps, core_ids=list(range(NCORES)), trace=_trace)
    y = np.empty((B, N, C), dtype=np.float32)
    for c in range(NCORES):
        o = np.asarray(res.results[c]["out"])
        for qh in range(2):
            base = qh * NH + c * 128
            for b in range(B):
                y[b, perms[b][base : base + 128]] = o[
                    :, qh * 256 + b * 128 : qh * 256 + (b + 1) * 128
                ].T
    if _trace:
        _cache["last_exec_time_ns"] = res.exec_time_ns
        _cache["last_profile"] = res.profile_json
    return y


# revision 21
# speedup vs baseline: 1.0922x; 1.0922x over previous
"""Trainium2 Bass kernel: 12-head attention (B=2, N=2048, C=768) on 8 NeuronCores.

Sharding: core c -> batch b = c // 4, head-group g = c % 4 (heads 3g..3g+2).

Key optimizations over the naive head-sharded layout:

- Mask compaction: the key mask is host-visible, so tokens of each batch are
  permuted so kept keys (~1002/1034 of 2048) come first; K/V/attention only
  process ceil(max_kept/128)*128 keys instead of 2048 (QK, exp, PV all shrink
  ~45%). Queries still cover all 2048 tokens; the host un-permutes at the end.
  Keys beyond the kept count are real (masked) tokens whose V rows and
  softmax-denominator column are zeroed via the mf vector, so they contribute
  exactly 0 to numerator and denominator (matching the reference).

- Head packing on the PE array: wq/wk are laid out [q0|q1] / [k0|k1] so head 0
  lives on SBUF partitions 0-63 and head 1 on 64-127. QK^T has contraction 64,
  so head-0/head-1 matmuls occupy disjoint row-halves of the 128x128 array
  (tile_position auto-derived from base partitions) and run concurrently, and
  their LDWEIGHTS overlap the other head's in-flight matmuls. Heads 0/1 are
  processed chunk-interleaved in one "pair unit" (two PSUM o-accumulators);
  head 2 uses duplicated columns [q2|q2] / [k2|k2] and alternates halves.

- The attention phase is scalar-engine(exp)-bound, so everything else hides
  under it: the kst[:,1]/qs2/V projections run as fill-in work inserted
  between attention chunks; attention starts as soon as the k0|k1 and
  half-0 q0|q1 projections land instead of after the full QKV phase. PV
  matmuls for chunk c are emitted after chunk c+1's QK so the FIFO tensor
  queue never head-of-line blocks on the exp.

- Softmax denominator via an extra all-ones column appended to V (one PV
  matmul yields values + row sums). Normalization never touches the scalar
  engine (DVE copy + gpsimd row-copy/broadcast + DVE reciprocal/multiply),
  and the PSUM accumulator is released after a single copy so the next
  unit's PV can start immediately (keeps the PE HAM-warm).

- Fully-useful 8-way AllToAll per query half: each half's 1024 tokens split
  into 8 blocks of 128; receiver j gets all 768 channels of token-block j for
  BOTH batches, so the projection runs on 256 columns (128 tokens x 2
  batches) per half with zero wasted wire bytes or flops. A2A#0 and proj#0
  hide under the second half's attention; during the exposed A2A#1 the PE
  runs proj#0 plus warm-up filler matmuls so the final projection executes
  at full clock. proj DMAs are sequenced before the next collective because
  DMAs issued after a collective barrier-wait on it.
"""

import numpy as np
import ml_dtypes

B, N, C = 2, 2048, 768
H, HD = 12, 64
HPG = 3            # heads per core
GPB = 4            # cores (head-groups) per batch
NCORES = 8
SCALE = float(HD) ** -0.5
DCH = C // 128     # 6 contraction chunks
NH = N // 2        # 1024 query tokens per half

bf = ml_dtypes.bfloat16

_cache = {}


def _build(nkch):
    import concourse.mybir as mybir
    import concourse.tile as tile
    from concourse import bacc

    fp32 = mybir.dt.float32
    bfl = mybir.dt.bfloat16
    EXP = mybir.ActivationFunctionType.Exp
    MULT = mybir.AluOpType.mult

    NK = nkch * 128  # padded key count

    nc = bacc.Bacc(None, num_devices=NCORES)
    xT = nc.declare_dram_parameter("xT", [C, N], bfl, isOutput=False)
    wq = nc.declare_dram_parameter("wq", [C, 256], bfl, isOutput=False)
    wk = nc.declare_dram_parameter("wk", [C, 256], bfl, isOutput=False)
    wv = nc.declare_dram_parameter("wv", [C, HPG * HD], bfl, isOutput=False)
    wp = nc.declare_dram_parameter("wp", [C, C], bfl, isOutput=False)
    bp = nc.declare_dram_parameter("bp", [128, DCH], fp32, isOutput=False)
    mf = nc.declare_dram_parameter("mf", [128, nkch], fp32, isOutput=False)
    out = nc.declare_dram_parameter("out", [C, 512], fp32, isOutput=True)

    with tile.TileContext(nc) as tc:
        with (
            tc.tile_pool(name="const", bufs=1) as cpool,
            tc.tile_pool(name="work", bufs=1) as wpool,
            tc.tile_pool(name="pp", bufs=2) as ppool,
        ):
            # ---------------- input loads (order = need order) ----------------
            mf_sb = cpool.tile([128, nkch], fp32, tag="mf")
            nc.sync.dma_start(mf_sb[:], mf[:])
            wk_sb = cpool.tile([128, DCH, 256], bfl, tag="wk")
            nc.sync.dma_start(wk_sb[:], wk.rearrange("(o p) c -> p o c", p=128))
            wq_sb = cpool.tile([128, DCH, 256], bfl, tag="wq")
            nc.sync.dma_start(wq_sb[:], wq.rearrange("(o p) c -> p o c", p=128))
            xT_sb = cpool.tile([128, DCH, N], bfl, tag="xT")
            xT_r = xT.rearrange("(o p) t -> p o t", p=128)
            nc.sync.dma_start(xT_sb[:, :, 0:512], xT_r[:, :, 0:512])
            wv_sb = cpool.tile([128, DCH, HPG * HD], bfl, tag="wv")
            nc.sync.dma_start(wv_sb[:], wv.rearrange("(o p) c -> p o c", p=128))
            for tq in range(1, 4):
                nc.sync.dma_start(
                    xT_sb[:, :, tq * 512 : (tq + 1) * 512],
                    xT_r[:, :, tq * 512 : (tq + 1) * 512],
                )
            wp_sb = cpool.tile([128, DCH, C], bfl, tag="wp")
            nc.sync.dma_start(wp_sb[:], wp.rearrange("(o p) c -> p o c", p=128))
            bp_sb = cpool.tile([128, DCH], fp32, tag="bp")
            nc.sync.dma_start(bp_sb[:], bp[:])

            # preload the exp table set while DMAs run
            warm = cpool.tile([1, 8], fp32, tag="warm")
            nc.vector.memset(warm[:], 0.0)
            nc.scalar.activation(warm[:], warm[:], EXP)
            ones_sb = cpool.tile([128, 64], bfl, tag="ones")
            nc.vector.memset(ones_sb[:], 1.0)

            qs = wpool.tile([128, N], bfl, tag="qs")      # [q0 | q1] channel-major
            qs2 = wpool.tile([128, N], bfl, tag="qs2")    # [q2 | q2]
            kst = wpool.tile([128, 2, NK], bfl, tag="kst")  # [:,0]=[k0|k1] [:,1]=[k2|k2]
            V3 = wpool.tile([128, nkch, HPG, HD + 1], bfl, tag="V3")

            # PSUM: tag "s" 2 slots x 2 banks (QK scores + all projection /
            # fill-in tiles), tag "o" 2 slots x 2 banks (live PV accumulators).
            sps_cm = tc.tile_pool(name="sps", bufs=2, space="PSUM")
            sps = sps_cm.__enter__()
            ops_cm = tc.tile_pool(name="ops", bufs=2, space="PSUM")
            ops = ops_cm.__enter__()

            def qk_pass(which, m, tq):
                """Q or K projection Mtile m over token quarter tq (512)."""
                lo = tq * 512
                w_sb = wq_sb if which == "q" else wk_sb
                wid = min(512, (NK - lo) if which == "k" else 512)
                if wid <= 0:
                    return
                t = sps.tile([128, NH], fp32, tag="s", name="qk_t")[:, :wid]
                for kk in range(DCH):
                    nc.tensor.matmul(
                        t[:],
                        lhsT=w_sb[:, kk, m * 128 : (m + 1) * 128],
                        rhs=xT_sb[:, kk, lo : lo + wid],
                        start=(kk == 0),
                        stop=(kk == DCH - 1),
                    )
                dst = (qs if m == 0 else qs2) if which == "q" else None
                if which == "q":
                    nc.vector.tensor_copy(dst[:, lo : lo + wid], t[:])
                else:
                    nc.vector.tensor_copy(kst[:, m, lo : lo + wid], t[:])

            def v_pass(c):
                """V projection for key chunk c -> V3 (values * mf, ones col)."""
                v_t = sps.tile([128, NH], fp32, tag="s", name="v_t")[:, : HPG * HD]
                for kk in range(DCH):
                    nc.tensor.matmul(
                        v_t[:],
                        lhsT=xT_sb[:, kk, c * 128 : (c + 1) * 128],
                        rhs=wv_sb[:, kk, :],
                        start=(kk == 0),
                        stop=(kk == DCH - 1),
                    )
                nc.vector.tensor_scalar_mul(
                    V3[:, c, :, 0:HD],
                    v_t[:].rearrange("p (h d) -> p h d", h=HPG),
                    mf_sb[:, c : c + 1],
                )
                nc.vector.tensor_copy(
                    V3[:, c, :, HD], mf_sb[:, c : c + 1].to_broadcast((128, HPG))
                )

            OnA = [wpool.tile([128, NH], bfl, tag=f"OnA{q}", name=f"OnA{q}") for q in range(2)]
            OnB = [wpool.tile([64, NH], bfl, tag=f"OnB{q}", name=f"OnB{q}") for q in range(2)]

            def normalize(h, qh, o_t):
                """osb <- o in bf16 (frees PSUM fast); denominator row is
                broadcast to 64 partitions via a rank-1 bf16 PE matmul
                (ones x row) so the gpsimd queue stays free for collective
                triggers; then DVE reciprocal + multiply."""
                osb = wpool.tile([HD + 1, NH], bfl, tag="osb", bufs=2, name="osb")
                nc.vector.tensor_copy(osb[:], o_t[:])
                rbb = sps.tile([128, NH], fp32, tag="s", name="rbb")[0:HD, :]
                for n2 in range(2):
                    nc.tensor.matmul(
                        rbb[:, n2 * 512 : (n2 + 1) * 512],
                        lhsT=ones_sb[HD : HD + 1, :],
                        rhs=osb[HD : HD + 1, n2 * 512 : (n2 + 1) * 512],
                        start=True,
                        stop=True,
                    )
                rb = wpool.tile([HD, NH], fp32, tag="rb", bufs=2, name="rb")
                nc.vector.reciprocal_approx_fast(rb[:], rbb[:])
                dst = OnA[qh][h * 64 : (h + 1) * 64, :] if h < 2 else OnB[qh][:, :]
                nc.vector.tensor_tensor(dst, osb[0:HD, :], rb[:], MULT)

            def qk_mm(s_t, ksrc, qsrc, base, c, qh):
                for n2 in range(2):
                    nc.tensor.matmul(
                        s_t[:, n2 * 512 : (n2 + 1) * 512],
                        lhsT=ksrc[base : base + 64, c * 128 : (c + 1) * 128],
                        rhs=qsrc[
                            base : base + 64,
                            qh * NH + n2 * 512 : qh * NH + (n2 + 1) * 512,
                        ],
                        start=True,
                        stop=True,
                    )

            def pv_mm(o_t, p_t, c, h):
                for n2 in range(2):
                    nc.tensor.matmul(
                        o_t[:, n2 * 512 : (n2 + 1) * 512],
                        lhsT=V3[:, c, h, :],
                        rhs=p_t[:, c, n2 * 512 : (n2 + 1) * 512],
                        start=(c == 0),
                        stop=(c == nkch - 1),
                    )

            def pair_unit(qh, extras=()):
                """Heads 0+1, chunk-interleaved, query half qh. PV for chunk
                c-1 is emitted after chunk c's QK (FIFO queue stays unblocked).
                extras: thunks inserted one per chunk (fill-in projections)."""
                p_t = [
                    ppool.tile([128, nkch, NH], bfl, tag="p", name=f"pu{h}")
                    for h in range(2)
                ]
                o_t = [ops.tile([HD + 1, NH], fp32, tag="o", name=f"ou{h}") for h in range(2)]
                ex = list(extras)
                for c in range(nkch):
                    if c < len(ex) and ex[c] is not None:
                        ex[c]()
                    s_t = []
                    for h in range(2):
                        st = sps.tile([128, NH], fp32, tag="s", name=f"s{h}")
                        qk_mm(st, kst[:, 0], qs, 64 * h, c, qh)
                        s_t.append(st)
                    for h in range(2):
                        nc.scalar.activation(p_t[h][:, c, :], s_t[h][:], EXP, scale=SCALE)
                    if c > 0:
                        for h in range(2):
                            pv_mm(o_t[h], p_t[h], c - 1, h)
                for h in range(2):
                    pv_mm(o_t[h], p_t[h], nkch - 1, h)
                for h in range(2):
                    normalize(h, qh, o_t[h])

            def h2_unit(qh, extras=()):
                """Head 2 over query half qh; kst[:,1]/qs2 hold [k2|k2]/[q2|q2]
                so chunks alternate array row-halves."""
                p_t = ppool.tile([128, nkch, NH], bfl, tag="p", name="pu2")
                o_t = ops.tile([HD + 1, NH], fp32, tag="o", name="ou2")
                ex = list(extras)
                for c in range(nkch):
                    if c < len(ex) and ex[c] is not None:
                        ex[c]()
                    s_t = sps.tile([128, NH], fp32, tag="s", name="s2")
                    qk_mm(s_t, kst[:, 1], qs2, 64 * (c % 2), c, qh)
                    nc.scalar.activation(p_t[:, c, :], s_t[:], EXP, scale=SCALE)
                    if c > 0:
                        pv_mm(o_t, p_t, c - 1, 2)
                pv_mm(o_t, p_t, nkch - 1, 2)
                normalize(2, qh, o_t)

            ag_in = [
                nc.dram_tensor(f"ag_in{q}", [NCORES * HPG * HD, 128], bfl)
                for q in range(2)
            ]
            ag_out = [
                nc.dram_tensor(f"ag_out{q}", [NCORES * HPG * HD, 128], bfl)
                for q in range(2)
            ]

            def bounce(qh):
                agi_r = ag_in[qh].rearrange("(j p) t -> p j t", j=NCORES)
                nc.sync.dma_start(
                    agi_r[0:128, :, :],
                    OnA[qh][:, :].rearrange("p (j t) -> p j t", j=NCORES),
                )
                nc.sync.dma_start(
                    agi_r[128:192, :, :],
                    OnB[qh][:, :].rearrange("p (j t) -> p j t", j=NCORES),
                )

            def a2a(qh):
                nc.gpsimd.collective_compute(
                    "AllToAll",
                    mybir.AluOpType.bypass,
                    replica_groups=[[0, 1, 2, 3, 4, 5, 6, 7]],
                    ins=[ag_in[qh][:].opt()],
                    outs=[ag_out[qh][:].opt()],
                )

            out_r = out.rearrange("(o p) t -> p o t", p=128)

            def proj_dma(qh):
                at_sb = wpool.tile(
                    [128, 2, DCH, 128], bfl, tag="at", bufs=2, name="at_sb"
                )
                nc.sync.dma_start(
                    at_sb[:], ag_out[qh].rearrange("(b o p) t -> p b o t", p=128, b=2)
                )
                return at_sb

            def proj_mtile(qh, at_sb, m):
                y_ps = sps.tile([128, NH], fp32, tag="s", name="y_ps")[:, :256]
                for kk in range(DCH):
                    nc.tensor.matmul(
                        y_ps[:].rearrange("p (b t) -> p b t", b=2),
                        lhsT=wp_sb[:, kk, m * 128 : (m + 1) * 128],
                        rhs=at_sb[:, :, kk, :],
                        start=(kk == 0),
                        stop=(kk == DCH - 1),
                    )
                y_sb = wpool.tile([128, 256], fp32, tag="y", bufs=2, name="y_sb")
                nc.vector.tensor_scalar_add(y_sb[:], y_ps[:], bp_sb[:, m : m + 1])
                nc.sync.dma_start(out_r[:, m, qh * 256 : (qh + 1) * 256], y_sb[:])

            def proj_pass(qh, at_sb):
                for m in range(DCH):
                    proj_mtile(qh, at_sb, m)

            def warm_fill(n):
                """Junk matmuls that keep the PE HAM-warm while waiting."""
                for _ in range(n):
                    w_ps = sps.tile([128, NH], fp32, tag="s", name="w_ps")[:, :256]
                    nc.tensor.matmul(
                        w_ps[:], lhsT=wp_sb[:, 0, 0:128], rhs=wp_sb[:, 1, 0:256],
                        start=True, stop=True,
                    )

            # ---------------- schedule ----------------
            # Minimal bootstrap so the first exp lands as early as possible:
            # keys/queries for the first chunks only; everything else becomes
            # fill-in work inside the scalar-engine-bound attention units.
            nq = (NK + 511) // 512  # K token-quarters (3 for NK=1152)
            qk_pass("k", 0, 0)
            qk_pass("q", 0, 0)
            qk_pass("q", 0, 1)
            for c in range(min(4, nkch)):
                v_pass(c)

            vs = [lambda c=c: v_pass(c) for c in range(4, nkch)]
            # pair(0) fill-ins with deadlines: v(c) at slot <= c, k0 quarter q
            # by chunk 4q, q2 (qs2) before h2_unit(0).
            ex_p0 = [lambda: qk_pass("q", 1, 0), lambda: qk_pass("q", 1, 1)]
            ex_p0 += vs[:1]
            ex_p0 += [lambda: qk_pass("k", 0, 1)]
            ex_p0 += vs[1:2]
            ex_p0 += [lambda q=q: qk_pass("k", 0, q) for q in range(2, nq)]
            ex_p0 += vs[2:]
            ex_h0 = [lambda q=q: qk_pass("k", 1, q) for q in range(2, nq)]
            ex_h0 += [lambda: qk_pass("q", 0, 2), lambda: qk_pass("q", 0, 3)]
            ex_p1 = [lambda: qk_pass("q", 1, 2), lambda: qk_pass("q", 1, 3)]

            # ---- half 0 (tokens 0-1023)
            pair_unit(0, extras=ex_p0[:nkch])
            for f in ex_p0[nkch:]:
                f()
            qk_pass("k", 1, 0)
            qk_pass("k", 1, 1)
            h2_unit(0, extras=ex_h0[:nkch])
            for f in ex_h0[nkch:]:
                f()
            bounce(0)
            a2a(0)

            # ---- half 1 (tokens 1024-2047); A2A#0 hides under pair(1) and
            # proj#0 runs as fill-in inside h2(1) (emission order = PE order).
            pair_unit(1, extras=ex_p1)
            at0 = proj_dma(0)
            ex_h1 = [None] * max(0, nkch - DCH) + [
                lambda m=m: proj_mtile(0, at0, m) for m in range(DCH)
            ]
            h2_unit(1, extras=ex_h1[:nkch])
            for f in ex_h1[nkch:]:
                if f is not None:
                    f()
            bounce(1)
            a2a(1)
            warm_fill(60)
            at1 = proj_dma(1)
            proj_pass(1, at1)

            ops_cm.__exit__(None, None, None)
            sps_cm.__exit__(None, None, None)

    nc.finalize()
    return nc


def _prep(x, mask, w_qkv, w_proj, b_proj):
    """Host-side compaction: per-batch token permutation (kept keys first) and
    per-core input shards."""
    perms, counts = [], []
    for b in range(B):
        perm = np.argsort(1 - mask[b], kind="stable")
        perms.append(perm)
        counts.append(int(mask[b].sum()))
    nkch = max(1, int(np.ceil(max(counts) / 128)))
    NK = nkch * 128

    xTs = []
    mfs = []
    for b in range(B):
        xp = np.ascontiguousarray(x[b][perms[b]].T).astype(bf)
        xTs.append(xp)
        m = np.zeros(NK, dtype=np.float32)
        m[: counts[b]] = 1.0
        mfs.append(np.ascontiguousarray(m.reshape(nkch, 128).T))

    bp_t = np.ascontiguousarray(b_proj.astype(np.float32).reshape(DCH, 128).T)
    wp_t = w_proj.astype(bf)

    in_maps = []
    for c in range(NCORES):
        b, g = c // GPB, c % GPB
        heads = [3 * g, 3 * g + 1, 3 * g + 2]
        q_cols = [h * HD + d for h in (heads[0], heads[1], heads[2], heads[2]) for d in range(HD)]
        k_cols = [C + h * HD + d for h in (heads[0], heads[1], heads[2], heads[2]) for d in range(HD)]
        v_cols = [2 * C + h * HD + d for h in heads for d in range(HD)]
        in_maps.append(
            {
                "xT": xTs[b],
                "wq": np.ascontiguousarray(w_qkv[:, q_cols]).astype(bf),
                "wk": np.ascontiguousarray(w_qkv[:, k_cols]).astype(bf),
                "wv": np.ascontiguousarray(w_qkv[:, v_cols]).astype(bf),
                "wp": wp_t,
                "bp": bp_t,
                "mf": mfs[b],
            }
        )
    return in_maps, perms, nkch


def kernel(x, mask, w_qkv, w_proj, b_proj, _trace=False):
    from concourse.bass_utils import run_bass_kernel_spmd

    x = np.asarray(x, dtype=np.float32)
    mask = np.asarray(mask)
    w_qkv = np.asarray(w_qkv, dtype=np.float32)
    w_proj = np.asarray(w_proj, dtype=np.float32)
    b_proj = np.asarray(b_proj, dtype=np.float32)
    in_maps, perms, nkch = _prep(x, mask, w_qkv, w_proj, b_proj)
    if ("nc", nkch) not in _cache:
        _cache[("nc", nkch)] = _build(nkch)
    nc = _cache[("nc", nkch)]
    res = run_bass_kernel_spmd(nc, in_maps, core_ids=list(range(NCORES)), trace=_trace)
    y = np.empty((B, N, C), dtype=np.float32)
    for c in range(NCORES):
        o = np.asarray(res.results[c]["out"])
        for qh in range(2):
            base = qh * NH + c * 128
            for b in range(B):
                y[b, perms[b][base : base + 128]] = o[
                    :, qh * 256 + b * 128 : qh * 256 + (b + 1) * 128
                ].T
    if _trace:
        _cache["last_exec_time_ns"] = res.exec_time_ns
        _cache["last_profile"] = res.profile_json
    return y


# revision 24
# speedup vs baseline: 1.1002x; 1.0074x over previous
"""Trainium2 Bass kernel: 12-head attention (B=2, N=2048, C=768) on 8 NeuronCores.

Sharding: core c -> batch b = c // 4, head-group g = c % 4 (heads 3g..3g+2).

Key optimizations over the naive head-sharded layout:

- Mask compaction: the key mask is host-visible, so tokens of each batch are
  permuted so kept keys (~1002/1034 of 2048) come first; K/V/attention only
  process ceil(max_kept/128)*128 keys instead of 2048 (QK, exp, PV all shrink
  ~45%). Queries still cover all 2048 tokens; the host un-permutes at the end.
  Keys beyond the kept count are real (masked) tokens whose V rows and
  softmax-denominator column are zeroed via the mf vector, so they contribute
  exactly 0 to numerator and denominator (matching the reference).

- Head packing on the PE array: wq/wk are laid out [q0|q1] / [k0|k1] so head 0
  lives on SBUF partitions 0-63 and head 1 on 64-127. QK^T has contraction 64,
  so head-0/head-1 matmuls occupy disjoint row-halves of the 128x128 array
  (tile_position auto-derived from base partitions) and run concurrently, and
  their LDWEIGHTS overlap the other head's in-flight matmuls. Heads 0/1 are
  processed chunk-interleaved in one "pair unit" (two PSUM o-accumulators);
  head 2 uses duplicated columns [q2|q2] / [k2|k2] and alternates halves.

- The attention phase is scalar-engine(exp)-bound, so everything else hides
  under it: the kst[:,1]/qs2/V projections run as fill-in work inserted
  between attention chunks; attention starts as soon as the k0|k1 and
  half-0 q0|q1 projections land instead of after the full QKV phase. PV
  matmuls for chunk c are emitted after chunk c+1's QK so the FIFO tensor
  queue never head-of-line blocks on the exp.

- Softmax denominator via an extra all-ones column appended to V (one PV
  matmul yields values + row sums). Normalization never touches the scalar
  engine (DVE copy + gpsimd row-copy/broadcast + DVE reciprocal/multiply),
  and the PSUM accumulator is released after a single copy so the next
  unit's PV can start immediately (keeps the PE HAM-warm).

- Fully-useful 8-way AllToAll per query half: each half's 1024 tokens split
  into 8 blocks of 128; receiver j gets all 768 channels of token-block j for
  BOTH batches, so the projection runs on 256 columns (128 tokens x 2
  batches) per half with zero wasted wire bytes or flops. A2A#0 and proj#0
  hide under the second half's attention; during the exposed A2A#1 the PE
  runs proj#0 plus warm-up filler matmuls so the final projection executes
  at full clock. proj DMAs are sequenced before the next collective because
  DMAs issued after a collective barrier-wait on it.
"""

import numpy as np
import ml_dtypes

B, N, C = 2, 2048, 768
H, HD = 12, 64
HPG = 3            # heads per core
GPB = 4            # cores (head-groups) per batch
NCORES = 8
SCALE = float(HD) ** -0.5
DCH = C // 128     # 6 contraction chunks
NH = N // 2        # 1024 query tokens per half

bf = ml_dtypes.bfloat16

_cache = {}


def _build(nkch):
    import concourse.mybir as mybir
    import concourse.tile as tile
    from concourse import bacc

    fp32 = mybir.dt.float32
    bfl = mybir.dt.bfloat16
    EXP = mybir.ActivationFunctionType.Exp
    MULT = mybir.AluOpType.mult

    NK = nkch * 128  # padded key count

    nc = bacc.Bacc(None, num_devices=NCORES)
    xT = nc.declare_dram_parameter("xT", [C, N], bfl, isOutput=False)
    wq = nc.declare_dram_parameter("wq", [C, 256], bfl, isOutput=False)
    wk = nc.declare_dram_parameter("wk", [C, 256], bfl, isOutput=False)
    wv = nc.declare_dram_parameter("wv", [C, HPG * HD], bfl, isOutput=False)
    wp = nc.declare_dram_parameter("wp", [C, C], bfl, isOutput=False)
    bp = nc.declare_dram_parameter("bp", [128, DCH], fp32, isOutput=False)
    mf = nc.declare_dram_parameter("mf", [128, nkch], fp32, isOutput=False)
    out = nc.declare_dram_parameter("out", [C, 512], fp32, isOutput=True)

    with tile.TileContext(nc) as tc:
        with (
            tc.tile_pool(name="const", bufs=1) as cpool,
            tc.tile_pool(name="work", bufs=1) as wpool,
            tc.tile_pool(name="pp", bufs=2) as ppool,
        ):
            # ---------------- input loads (order = need order) ----------------
            mf_sb = cpool.tile([128, nkch], fp32, tag="mf")
            nc.sync.dma_start(mf_sb[:], mf[:])
            wk_sb = cpool.tile([128, DCH, 256], bfl, tag="wk")
            nc.sync.dma_start(wk_sb[:], wk.rearrange("(o p) c -> p o c", p=128))
            wq_sb = cpool.tile([128, DCH, 256], bfl, tag="wq")
            nc.sync.dma_start(wq_sb[:], wq.rearrange("(o p) c -> p o c", p=128))
            xT_sb = cpool.tile([128, DCH, N], bfl, tag="xT")
            xT_r = xT.rearrange("(o p) t -> p o t", p=128)
            nc.sync.dma_start(xT_sb[:, :, 0:512], xT_r[:, :, 0:512])
            wv_sb = cpool.tile([128, DCH, HPG * HD], bfl, tag="wv")
            nc.sync.dma_start(wv_sb[:], wv.rearrange("(o p) c -> p o c", p=128))
            for tq in range(1, 4):
                nc.sync.dma_start(
                    xT_sb[:, :, tq * 512 : (tq + 1) * 512],
                    xT_r[:, :, tq * 512 : (tq + 1) * 512],
                )
            wp_sb = cpool.tile([128, DCH, C], bfl, tag="wp")
            nc.sync.dma_start(wp_sb[:], wp.rearrange("(o p) c -> p o c", p=128))
            bp_sb = cpool.tile([128, DCH], fp32, tag="bp")
            nc.sync.dma_start(bp_sb[:], bp[:])

            # preload the exp table set while DMAs run
            warm = cpool.tile([1, 8], fp32, tag="warm")
            nc.vector.memset(warm[:], 0.0)
            nc.scalar.activation(warm[:], warm[:], EXP)
            ones_sb = cpool.tile([128, 64], bfl, tag="ones")
            nc.vector.memset(ones_sb[:], 1.0)

            qs = wpool.tile([128, N], bfl, tag="qs")      # [q0 | q1] channel-major
            qs2 = wpool.tile([128, N], bfl, tag="qs2")    # [q2 | q2]
            kst = wpool.tile([128, 2, NK], bfl, tag="kst")  # [:,0]=[k0|k1] [:,1]=[k2|k2]
            V3 = wpool.tile([128, nkch, HPG, HD + 1], bfl, tag="V3")

            # PSUM: tag "s" 2 slots x 2 banks (QK scores + all projection /
            # fill-in tiles), tag "o" 2 slots x 2 banks (live PV accumulators).
            sps_cm = tc.tile_pool(name="sps", bufs=2, space="PSUM")
            sps = sps_cm.__enter__()
            ops_cm = tc.tile_pool(name="ops", bufs=2, space="PSUM")
            ops = ops_cm.__enter__()

            def qk_pass(which, m, tq):
                """Q or K projection Mtile m over token quarter tq (512)."""
                lo = tq * 512
                w_sb = wq_sb if which == "q" else wk_sb
                wid = min(512, (NK - lo) if which == "k" else 512)
                if wid <= 0:
                    return
                t = sps.tile([128, NH], fp32, tag="s", name="qk_t")[:, :wid]
                for kk in range(DCH):
                    nc.tensor.matmul(
                        t[:],
                        lhsT=w_sb[:, kk, m * 128 : (m + 1) * 128],
                        rhs=xT_sb[:, kk, lo : lo + wid],
                        start=(kk == 0),
                        stop=(kk == DCH - 1),
                    )
                dst = (qs if m == 0 else qs2) if which == "q" else None
                if which == "q":
                    nc.vector.tensor_copy(dst[:, lo : lo + wid], t[:])
                else:
                    nc.vector.tensor_copy(kst[:, m, lo : lo + wid], t[:])

            def v_pass(c):
                """V projection for key chunk c -> V3 (values * mf, ones col)."""
                v_t = sps.tile([128, NH], fp32, tag="s", name="v_t")[:, : HPG * HD]
                for kk in range(DCH):
                    nc.tensor.matmul(
                        v_t[:],
                        lhsT=xT_sb[:, kk, c * 128 : (c + 1) * 128],
                        rhs=wv_sb[:, kk, :],
                        start=(kk == 0),
                        stop=(kk == DCH - 1),
                    )
                nc.vector.tensor_scalar_mul(
                    V3[:, c, :, 0:HD],
                    v_t[:].rearrange("p (h d) -> p h d", h=HPG),
                    mf_sb[:, c : c + 1],
                )
                nc.vector.tensor_copy(
                    V3[:, c, :, HD], mf_sb[:, c : c + 1].to_broadcast((128, HPG))
                )

            OnA = [wpool.tile([128, NH], bfl, tag=f"OnA{q}", name=f"OnA{q}") for q in range(2)]
            OnB = [wpool.tile([64, NH], bfl, tag=f"OnB{q}", name=f"OnB{q}") for q in range(2)]

            def normalize(h, qh, o_t):
                """osb <- o in bf16 (frees PSUM fast); denominator row is
                broadcast to 64 partitions via a rank-1 bf16 PE matmul
                (ones x row) so the gpsimd queue stays free for collective
                triggers; then DVE reciprocal + multiply."""
                osb = wpool.tile([HD + 1, NH], bfl, tag="osb", bufs=2, name="osb")
                nc.vector.tensor_copy(osb[:], o_t[:])
                rbb = sps.tile([128, NH], fp32, tag="s", name="rbb")[0:HD, :]
                for n2 in range(2):
                    nc.tensor.matmul(
                        rbb[:, n2 * 512 : (n2 + 1) * 512],
                        lhsT=ones_sb[HD : HD + 1, :],
                        rhs=osb[HD : HD + 1, n2 * 512 : (n2 + 1) * 512],
                        start=True,
                        stop=True,
                    )
                rb = wpool.tile([HD, NH], fp32, tag="rb", bufs=2, name="rb")
                nc.vector.reciprocal_approx_fast(rb[:], rbb[:])
                dst = OnA[qh][h * 64 : (h + 1) * 64, :] if h < 2 else OnB[qh][:, :]
                nc.vector.tensor_tensor(dst, osb[0:HD, :], rb[:], MULT)

            def qk_mm(s_t, ksrc, qsrc, base, c, qh):
                for n2 in range(2):
                    nc.tensor.matmul(
                        s_t[:, n2 * 512 : (n2 + 1) * 512],
                        lhsT=ksrc[base : base + 64, c * 128 : (c + 1) * 128],
                        rhs=qsrc[
                            base : base + 64,
                            qh * NH + n2 * 512 : qh * NH + (n2 + 1) * 512,
                        ],
                        start=True,
                        stop=True,
                    )

            def pv_mm(o_t, p_t, c, h):
                for n2 in range(2):
                    nc.tensor.matmul(
                        o_t[:, n2 * 512 : (n2 + 1) * 512],
                        lhsT=V3[:, c, h, :],
                        rhs=p_t[:, c, n2 * 512 : (n2 + 1) * 512],
                        start=(c == 0),
                        stop=(c == nkch - 1),
                    )

            def pair_unit(qh, extras=()):
                """Heads 0+1, chunk-interleaved, query half qh. PV for chunk
                c-1 is emitted after chunk c's QK (FIFO queue stays unblocked).
                extras: thunks inserted one per chunk (fill-in projections)."""
                p_t = [
                    ppool.tile([128, nkch, NH], bfl, tag="p", name=f"pu{h}")
                    for h in range(2)
                ]
                o_t = [ops.tile([HD + 1, NH], fp32, tag="o", name=f"ou{h}") for h in range(2)]
                ex = list(extras)
                for c in range(nkch):
                    if c < len(ex) and ex[c] is not None:
                        ex[c]()
                    s_t = []
                    for h in range(2):
                        st = sps.tile([128, NH], fp32, tag="s", name=f"s{h}")
                        qk_mm(st, kst[:, 0], qs, 64 * h, c, qh)
                        s_t.append(st)
                    for h in range(2):
                        nc.scalar.activation(p_t[h][:, c, :], s_t[h][:], EXP, scale=SCALE)
                    if c > 0:
                        for h in range(2):
                            pv_mm(o_t[h], p_t[h], c - 1, h)
                for h in range(2):
                    pv_mm(o_t[h], p_t[h], nkch - 1, h)
                for h in range(2):
                    normalize(h, qh, o_t[h])

            def h2_unit(qh, extras=()):
                """Head 2 over query half qh; kst[:,1]/qs2 hold [k2|k2]/[q2|q2]
                so chunks alternate array row-halves."""
                p_t = ppool.tile([128, nkch, NH], bfl, tag="p", name="pu2")
                o_t = ops.tile([HD + 1, NH], fp32, tag="o", name="ou2")
                ex = list(extras)
                for c in range(nkch):
                    if c < len(ex) and ex[c] is not None:
                        ex[c]()
                    s_t = sps.tile([128, NH], fp32, tag="s", name="s2")
                    qk_mm(s_t, kst[:, 1], qs2, 64 * (c % 2), c, qh)
                    nc.scalar.activation(p_t[:, c, :], s_t[:], EXP, scale=SCALE)
                    if c > 0:
                        pv_mm(o_t, p_t, c - 1, 2)
                pv_mm(o_t, p_t, nkch - 1, 2)
                normalize(2, qh, o_t)

            ag_in = [
                nc.dram_tensor(f"ag_in{q}", [NCORES * HPG * HD, 128], bfl)
                for q in range(2)
            ]
            ag_out = [
                nc.dram_tensor(f"ag_out{q}", [NCORES * HPG * HD, 128], bfl)
                for q in range(2)
            ]

            def bounce_a(qh):
                agi_r = ag_in[qh].rearrange("(j p) t -> p j t", j=NCORES)
                nc.sync.dma_start(
                    agi_r[0:128, :, :],
                    OnA[qh][:, :].rearrange("p (j t) -> p j t", j=NCORES),
                )

            def bounce_b(qh):
                agi_r = ag_in[qh].rearrange("(j p) t -> p j t", j=NCORES)
                nc.sync.dma_start(
                    agi_r[128:192, :, :],
                    OnB[qh][:, :].rearrange("p (j t) -> p j t", j=NCORES),
                )

            def a2a(qh):
                nc.gpsimd.collective_compute(
                    "AllToAll",
                    mybir.AluOpType.bypass,
                    replica_groups=[[0, 1, 2, 3, 4, 5, 6, 7]],
                    ins=[ag_in[qh][:].opt()],
                    outs=[ag_out[qh][:].opt()],
                )

            out_r = out.rearrange("(o p) t -> p o t", p=128)

            def proj_dma(qh):
                at_sb = wpool.tile(
                    [128, 2, DCH, 128], bfl, tag="at", bufs=2, name="at_sb"
                )
                nc.sync.dma_start(
                    at_sb[:], ag_out[qh].rearrange("(b o p) t -> p b o t", p=128, b=2)
                )
                return at_sb

            def proj_mtile(qh, at_sb, m):
                y_ps = sps.tile([128, NH], fp32, tag="s", name="y_ps")[:, :256]
                for kk in range(DCH):
                    nc.tensor.matmul(
                        y_ps[:].rearrange("p (b t) -> p b t", b=2),
                        lhsT=wp_sb[:, kk, m * 128 : (m + 1) * 128],
                        rhs=at_sb[:, :, kk, :],
                        start=(kk == 0),
                        stop=(kk == DCH - 1),
                    )
                y_sb = wpool.tile([128, 256], fp32, tag="y", bufs=4, name="y_sb")
                nc.vector.tensor_scalar_add(y_sb[:], y_ps[:], bp_sb[:, m : m + 1])
                nc.sync.dma_start(out_r[:, m, qh * 256 : (qh + 1) * 256], y_sb[:])

            def proj_pass(qh, at_sb):
                for m in range(DCH):
                    proj_mtile(qh, at_sb, m)

            def warm_fill(n):
                """Junk matmuls that keep the PE HAM-warm while waiting."""
                for _ in range(n):
                    w_ps = sps.tile([128, NH], fp32, tag="s", name="w_ps")[:, :256]
                    nc.tensor.matmul(
                        w_ps[:], lhsT=wp_sb[:, 0, 0:128], rhs=wp_sb[:, 1, 0:256],
                        start=True, stop=True,
                    )

            # ---------------- schedule ----------------
            # Minimal bootstrap so the first exp lands as early as possible:
            # keys/queries for the first chunks only; everything else becomes
            # fill-in work inside the scalar-engine-bound attention units.
            nq = (NK + 511) // 512  # K token-quarters (3 for NK=1152)
            qk_pass("k", 0, 0)
            qk_pass("q", 0, 0)
            qk_pass("q", 0, 1)
            for c in range(min(4, nkch)):
                v_pass(c)

            vs = [lambda c=c: v_pass(c) for c in range(4, nkch)]
            # pair(0) fill-ins with deadlines: v(c) at slot <= c, k0 quarter q
            # by chunk 4q, q2 (qs2) before h2_unit(0).
            ex_p0 = [lambda: qk_pass("q", 1, 0), lambda: qk_pass("q", 1, 1)]
            ex_p0 += vs[:1]
            ex_p0 += [lambda: qk_pass("k", 0, 1)]
            ex_p0 += vs[1:2]
            ex_p0 += [lambda q=q: qk_pass("k", 0, q) for q in range(2, nq)]
            ex_p0 += vs[2:]
            ex_h0 = [lambda q=q: qk_pass("k", 1, q) for q in range(2, nq)]
            ex_h0 += [lambda: qk_pass("q", 0, 2), lambda: qk_pass("q", 0, 3)]
            ex_p1 = [lambda: qk_pass("q", 1, 2), lambda: qk_pass("q", 1, 3)]

            # ---- half 0 (tokens 0-1023). The OnA bounce fires as soon as the
            # pair unit's normalize lands; the collective triggers right after
            # h2's (so every core arrives at the A2A barrier early).
            pair_unit(0, extras=ex_p0[:nkch])
            for f in ex_p0[nkch:]:
                f()
            bounce_a(0)
            qk_pass("k", 1, 0)
            qk_pass("k", 1, 1)
            h2_unit(0, extras=ex_h0[:nkch])
            for f in ex_h0[nkch:]:
                f()
            bounce_b(0)
            a2a(0)

            # ---- half 1 (tokens 1024-2047); A2A#0 hides under pair(1) and
            # proj#0 runs as fill-in inside h2(1) (emission order = PE order).
            pair_unit(1, extras=ex_p1)
            bounce_a(1)
            at0 = proj_dma(0)
            ex_h1 = [None] * max(0, nkch - DCH) + [
                lambda m=m: proj_mtile(0, at0, m) for m in range(DCH)
            ]
            h2_unit(1, extras=ex_h1[:nkch])
            for f in ex_h1[nkch:]:
                if f is not None:
                    f()
            bounce_b(1)
            a2a(1)
            warm_fill(60)
            at1 = proj_dma(1)
            proj_pass(1, at1)

            ops_cm.__exit__(None, None, None)
            sps_cm.__exit__(None, None, None)

    nc.finalize()
    return nc


def _prep(x, mask, w_qkv, w_proj, b_proj):
    """Host-side compaction: per-batch token permutation (kept keys first) and
    per-core input shards."""
    perms, counts = [], []
    for b in range(B):
        perm = np.argsort(1 - mask[b], kind="stable")
        perms.append(perm)
        counts.append(int(mask[b].sum()))
    nkch = max(1, int(np.ceil(max(counts) / 128)))
    NK = nkch * 128

    xTs = []
    mfs = []
    for b in range(B):
        xp = np.ascontiguousarray(x[b][perms[b]].T).astype(bf)
        xTs.append(xp)
        m = np.zeros(NK, dtype=np.float32)
        m[: counts[b]] = 1.0
        mfs.append(np.ascontiguousarray(m.reshape(nkch, 128).T))

    bp_t = np.ascontiguousarray(b_proj.astype(np.float32).reshape(DCH, 128).T)
    wp_t = w_proj.astype(bf)

    in_maps = []
    for c in range(NCORES):
        b, g = c // GPB, c % GPB
        heads = [3 * g, 3 * g + 1, 3 * g + 2]
        q_cols = [h * HD + d for h in (heads[0], heads[1], heads[2], heads[2]) for d in range(HD)]
        k_cols = [C + h * HD + d for h in (heads[0], heads[1], heads[2], heads[2]) for d in range(HD)]
        v_cols = [2 * C + h * HD + d for h in heads for d in range(HD)]
        in_maps.append(
            {
                "xT": xTs[b],
                "wq": np.ascontiguousarray(w_qkv[:, q_cols]).astype(bf),
                "wk": np.ascontiguousarray(w_qkv[:, k_cols]).astype(bf),
                "wv": np.ascontiguousarray(w_qkv[:, v_cols]).astype(bf),
                "wp": wp_t,
                "bp": bp_t,
                "mf": mfs[b],
            }
        )
    return in_maps, perms, nkch


def kernel(x, mask, w_qkv, w_proj, b_proj, _trace=False):
    from concourse.bass_utils import run_bass_kernel_spmd

    x = np.asarray(x, dtype=np.float32)
    mask = np.asarray(mask)
    w_qkv = np.asarray(w_qkv, dtype=np.float32)
    w_proj = np.asarray(w_proj, dtype=np.float32)
    b_proj = np.asarray(b_proj, dtype=np.float32)
    in_maps, perms, nkch = _prep(x, mask, w_qkv, w_proj, b_proj)
    if ("nc", nkch) not in _cache:
        _cache[("nc", nkch)] = _build(nkch)
    nc = _cache[("nc", nkch)]
    res = run_bass_kernel_spmd(nc, in_maps, core_ids=list(range(NCORES)), trace=_trace)
    y = np.empty((B, N, C), dtype=np.float32)
    for c in range(NCORES):
        o = np.asarray(res.results[c]["out"])
        for qh in range(2):
            base = qh * NH + c * 128
            for b in range(B):
                y[b, perms[b][base : base + 128]] = o[
                    :, qh * 256 + b * 128 : qh * 256 + (b + 1) * 128
                ].T
    if _trace:
        _cache["last_exec_time_ns"] = res.exec_time_ns
        _cache["last_profile"] = res.profile_json
    return y


# revision 28
# speedup vs baseline: 1.1114x; 1.0102x over previous
"""Trainium2 Bass kernel: 12-head attention (B=2, N=2048, C=768) on 8 NeuronCores.

Sharding: core c -> batch b = c // 4, head-group g = c % 4 (heads 3g..3g+2).

Key optimizations over the naive head-sharded layout:

- Mask compaction: the key mask is host-visible, so tokens of each batch are
  permuted so kept keys (~1002/1034 of 2048) come first; K/V/attention only
  process ceil(max_kept/128)*128 keys instead of 2048 (QK, exp, PV all shrink
  ~45%). Queries still cover all 2048 tokens; the host un-permutes at the end.
  Keys beyond the kept count are real (masked) tokens whose V rows and
  softmax-denominator column are zeroed via the mf vector, so they contribute
  exactly 0 to numerator and denominator (matching the reference).

- Head packing on the PE array: wq/wk are laid out [q0|q1] / [k0|k1] so head 0
  lives on SBUF partitions 0-63 and head 1 on 64-127. QK^T has contraction 64,
  so head-0/head-1 matmuls occupy disjoint row-halves of the 128x128 array
  (tile_position auto-derived from base partitions) and run concurrently, and
  their LDWEIGHTS overlap the other head's in-flight matmuls. Heads 0/1 are
  processed chunk-interleaved in one "pair unit" (two PSUM o-accumulators);
  head 2 uses duplicated columns [q2|q2] / [k2|k2] and alternates halves.

- The attention phase is scalar-engine(exp)-bound, so everything else hides
  under it: the kst[:,1]/qs2/V projections run as fill-in work inserted
  between attention chunks; attention starts as soon as the k0|k1 and
  half-0 q0|q1 projections land instead of after the full QKV phase. PV
  matmuls for chunk c are emitted after chunk c+1's QK so the FIFO tensor
  queue never head-of-line blocks on the exp.

- Softmax denominator via an extra all-ones column appended to V (one PV
  matmul yields values + row sums). Normalization never touches the scalar
  engine (DVE copy + gpsimd row-copy/broadcast + DVE reciprocal/multiply),
  and the PSUM accumulator is released after a single copy so the next
  unit's PV can start immediately (keeps the PE HAM-warm).

- Fully-useful 8-way AllToAll per query half: each half's 1024 tokens split
  into 8 blocks of 128; receiver j gets all 768 channels of token-block j for
  BOTH batches, so the projection runs on 256 columns (128 tokens x 2
  batches) per half with zero wasted wire bytes or flops. A2A#0 and proj#0
  hide under the second half's attention; during the exposed A2A#1 the PE
  runs proj#0 plus warm-up filler matmuls so the final projection executes
  at full clock. proj DMAs are sequenced before the next collective because
  DMAs issued after a collective barrier-wait on it.
"""

import numpy as np
import ml_dtypes

B, N, C = 2, 2048, 768
H, HD = 12, 64
HPG = 3            # heads per core
GPB = 4            # cores (head-groups) per batch
NCORES = 8
SCALE = float(HD) ** -0.5
DCH = C // 128     # 6 contraction chunks
NH = N // 2        # 1024 query tokens per half

bf = ml_dtypes.bfloat16

_cache = {}


def _build(nkch):
    import concourse.mybir as mybir
    import concourse.tile as tile
    from concourse import bacc

    fp32 = mybir.dt.float32
    bfl = mybir.dt.bfloat16
    EXP = mybir.ActivationFunctionType.Exp
    MULT = mybir.AluOpType.mult

    NK = nkch * 128  # padded key count

    nc = bacc.Bacc(None, num_devices=NCORES)
    xT = nc.declare_dram_parameter("xT", [C, N], bfl, isOutput=False)
    wq = nc.declare_dram_parameter("wq", [C, 256], bfl, isOutput=False)
    wk = nc.declare_dram_parameter("wk", [C, 256], bfl, isOutput=False)
    wv = nc.declare_dram_parameter("wv", [C, HPG * HD], bfl, isOutput=False)
    wp = nc.declare_dram_parameter("wp", [C, C], bfl, isOutput=False)
    bp = nc.declare_dram_parameter("bp", [128, DCH], fp32, isOutput=False)
    mf = nc.declare_dram_parameter("mf", [128, nkch], fp32, isOutput=False)
    out = nc.declare_dram_parameter("out", [C, 512], fp32, isOutput=True)

    with tile.TileContext(nc) as tc:
        with (
            tc.tile_pool(name="const", bufs=1) as cpool,
            tc.tile_pool(name="work", bufs=1) as wpool,
            tc.tile_pool(name="pp", bufs=2) as ppool,
        ):
            # ---------------- input loads (order = need order) ----------------
            mf_sb = cpool.tile([128, nkch], fp32, tag="mf")
            nc.sync.dma_start(mf_sb[:], mf[:])
            wk_sb = cpool.tile([128, DCH, 256], bfl, tag="wk")
            nc.sync.dma_start(wk_sb[:], wk.rearrange("(o p) c -> p o c", p=128))
            wq_sb = cpool.tile([128, DCH, 256], bfl, tag="wq")
            nc.sync.dma_start(wq_sb[:], wq.rearrange("(o p) c -> p o c", p=128))
            xT_sb = cpool.tile([128, DCH, N], bfl, tag="xT")
            xT_r = xT.rearrange("(o p) t -> p o t", p=128)
            nc.sync.dma_start(xT_sb[:, :, 0:512], xT_r[:, :, 0:512])
            wv_sb = cpool.tile([128, DCH, HPG * HD], bfl, tag="wv")
            nc.sync.dma_start(wv_sb[:], wv.rearrange("(o p) c -> p o c", p=128))
            for tq in range(1, 4):
                nc.sync.dma_start(
                    xT_sb[:, :, tq * 512 : (tq + 1) * 512],
                    xT_r[:, :, tq * 512 : (tq + 1) * 512],
                )
            wp_sb = cpool.tile([128, DCH, C], bfl, tag="wp")
            nc.sync.dma_start(wp_sb[:], wp.rearrange("(o p) c -> p o c", p=128))
            bp_sb = cpool.tile([128, DCH], fp32, tag="bp")
            nc.sync.dma_start(bp_sb[:], bp[:])

            # preload the exp table set while DMAs run
            warm = cpool.tile([1, 8], fp32, tag="warm")
            nc.vector.memset(warm[:], 0.0)
            nc.scalar.activation(warm[:], warm[:], EXP)
            ones_sb = cpool.tile([128, 64], bfl, tag="ones")
            nc.vector.memset(ones_sb[:], 1.0)

            qs = wpool.tile([128, N], bfl, tag="qs")      # [q0 | q1] channel-major
            qs2 = wpool.tile([128, N], bfl, tag="qs2")    # [q2 | q2]
            kst = wpool.tile([128, 2, NK], bfl, tag="kst")  # [:,0]=[k0|k1] [:,1]=[k2|k2]
            V3 = wpool.tile([128, nkch, HPG, HD + 1], bfl, tag="V3")

            # PSUM: tag "s" 2 slots x 2 banks (QK scores + all projection /
            # fill-in tiles), tag "o" 2 slots x 2 banks (live PV accumulators).
            sps_cm = tc.tile_pool(name="sps", bufs=2, space="PSUM")
            sps = sps_cm.__enter__()
            ops_cm = tc.tile_pool(name="ops", bufs=2, space="PSUM")
            ops = ops_cm.__enter__()

            def qk_pass(which, m, tq):
                """Q or K projection Mtile m over token quarter tq (512)."""
                lo = tq * 512
                w_sb = wq_sb if which == "q" else wk_sb
                wid = min(512, (NK - lo) if which == "k" else 512)
                if wid <= 0:
                    return
                t = sps.tile([128, NH], fp32, tag="s", name="qk_t")[:, :wid]
                for kk in range(DCH):
                    nc.tensor.matmul(
                        t[:],
                        lhsT=w_sb[:, kk, m * 128 : (m + 1) * 128],
                        rhs=xT_sb[:, kk, lo : lo + wid],
                        start=(kk == 0),
                        stop=(kk == DCH - 1),
                    )
                dst = (qs if m == 0 else qs2) if which == "q" else None
                if which == "q":
                    nc.vector.tensor_copy(dst[:, lo : lo + wid], t[:])
                else:
                    nc.vector.tensor_copy(kst[:, m, lo : lo + wid], t[:])

            def v_pass(c):
                """V projection for key chunk c -> V3 (values * mf, ones col)."""
                v_t = sps.tile([128, NH], fp32, tag="s", name="v_t")[:, : HPG * HD]
                for kk in range(DCH):
                    nc.tensor.matmul(
                        v_t[:],
                        lhsT=xT_sb[:, kk, c * 128 : (c + 1) * 128],
                        rhs=wv_sb[:, kk, :],
                        start=(kk == 0),
                        stop=(kk == DCH - 1),
                    )
                nc.vector.tensor_scalar_mul(
                    V3[:, c, :, 0:HD],
                    v_t[:].rearrange("p (h d) -> p h d", h=HPG),
                    mf_sb[:, c : c + 1],
                )
                nc.vector.tensor_copy(
                    V3[:, c, :, HD], mf_sb[:, c : c + 1].to_broadcast((128, HPG))
                )

            OnA = [wpool.tile([128, NH], bfl, tag=f"OnA{q}", name=f"OnA{q}") for q in range(2)]
            OnB = [wpool.tile([64, NH], bfl, tag=f"OnB{q}", name=f"OnB{q}") for q in range(2)]

            def normalize(h, qh, o_t):
                """osb <- o in bf16 (frees PSUM fast); denominator row is
                broadcast to 64 partitions via a rank-1 bf16 PE matmul
                (ones x row) so the gpsimd queue stays free for collective
                triggers; then DVE reciprocal + multiply."""
                osb = wpool.tile([HD + 1, NH], bfl, tag="osb", bufs=2, name="osb")
                nc.vector.tensor_copy(osb[:], o_t[:])
                rbb = sps.tile([128, NH], fp32, tag="s", name="rbb")[0:HD, :]
                for n2 in range(2):
                    nc.tensor.matmul(
                        rbb[:, n2 * 512 : (n2 + 1) * 512],
                        lhsT=ones_sb[HD : HD + 1, :],
                        rhs=osb[HD : HD + 1, n2 * 512 : (n2 + 1) * 512],
                        start=True,
                        stop=True,
                    )
                rb = wpool.tile([HD, NH], fp32, tag="rb", bufs=2, name="rb")
                nc.vector.reciprocal_approx_fast(rb[:], rbb[:])
                dst = OnA[qh][h * 64 : (h + 1) * 64, :] if h < 2 else OnB[qh][:, :]
                nc.vector.tensor_tensor(dst, osb[0:HD, :], rb[:], MULT)

            def qk_mm(s_t, ksrc, qsrc, base, c, qh):
                for n2 in range(2):
                    nc.tensor.matmul(
                        s_t[:, n2 * 512 : (n2 + 1) * 512],
                        lhsT=ksrc[base : base + 64, c * 128 : (c + 1) * 128],
                        rhs=qsrc[
                            base : base + 64,
                            qh * NH + n2 * 512 : qh * NH + (n2 + 1) * 512,
                        ],
                        start=True,
                        stop=True,
                    )

            def pv_mm(o_t, p_t, c, h):
                for n2 in range(2):
                    nc.tensor.matmul(
                        o_t[:, n2 * 512 : (n2 + 1) * 512],
                        lhsT=V3[:, c, h, :],
                        rhs=p_t[:, c, n2 * 512 : (n2 + 1) * 512],
                        start=(c == 0),
                        stop=(c == nkch - 1),
                    )

            def pair_unit(qh, extras=()):
                """Heads 0+1, chunk-interleaved, query half qh. PV for chunk
                c-1 is emitted after chunk c's QK (FIFO queue stays unblocked).
                extras: thunks inserted one per chunk (fill-in projections)."""
                p_t = [
                    ppool.tile([128, nkch, NH], bfl, tag="p", name=f"pu{h}")
                    for h in range(2)
                ]
                o_t = [ops.tile([HD + 1, NH], fp32, tag="o", name=f"ou{h}") for h in range(2)]
                ex = list(extras)
                for c in range(nkch):
                    if c < len(ex) and ex[c] is not None:
                        ex[c]()
                    s_t = []
                    for h in range(2):
                        st = sps.tile([128, NH], fp32, tag="s", name=f"s{h}")
                        qk_mm(st, kst[:, 0], qs, 64 * h, c, qh)
                        s_t.append(st)
                    for h in range(2):
                        nc.scalar.activation(p_t[h][:, c, :], s_t[h][:], EXP, scale=SCALE)
                    if c > 0:
                        for h in range(2):
                            pv_mm(o_t[h], p_t[h], c - 1, h)
                for h in range(2):
                    pv_mm(o_t[h], p_t[h], nkch - 1, h)
                for h in range(2):
                    normalize(h, qh, o_t[h])

            def h2_unit(qh, extras=()):
                """Head 2 over query half qh; kst[:,1]/qs2 hold [k2|k2]/[q2|q2]
                so chunks alternate array row-halves."""
                p_t = ppool.tile([128, nkch, NH], bfl, tag="p", name="pu2")
                o_t = ops.tile([HD + 1, NH], fp32, tag="o", name="ou2")
                ex = list(extras)
                for c in range(nkch):
                    if c < len(ex) and ex[c] is not None:
                        ex[c]()
                    s_t = sps.tile([128, NH], fp32, tag="s", name="s2")
                    qk_mm(s_t, kst[:, 1], qs2, 64 * (c % 2), c, qh)
                    nc.scalar.activation(p_t[:, c, :], s_t[:], EXP, scale=SCALE)
                    if c > 0:
                        pv_mm(o_t, p_t, c - 1, 2)
                pv_mm(o_t, p_t, nkch - 1, 2)
                normalize(2, qh, o_t)

            ag_a_in = [
                nc.dram_tensor(f"ag_a_in{q}", [NCORES * 128, 128], bfl)
                for q in range(2)
            ]
            ag_a_out = [
                nc.dram_tensor(f"ag_a_out{q}", [NCORES * 128, 128], bfl)
                for q in range(2)
            ]
            ag_b_in = [
                nc.dram_tensor(f"ag_b_in{q}", [NCORES * 64, 128], bfl)
                for q in range(2)
            ]
            ag_b_out = [
                nc.dram_tensor(f"ag_b_out{q}", [NCORES * 64, 128], bfl)
                for q in range(2)
            ]

            def bounce_a(qh):
                nc.sync.dma_start(
                    ag_a_in[qh].rearrange("(j p) t -> p j t", j=NCORES),
                    OnA[qh][:, :].rearrange("p (j t) -> p j t", j=NCORES),
                )

            def bounce_b(qh):
                nc.sync.dma_start(
                    ag_b_in[qh].rearrange("(j p) t -> p j t", j=NCORES),
                    OnB[qh][:, :].rearrange("p (j t) -> p j t", j=NCORES),
                )

            def a2a(qh, part):
                agi, ago = (ag_a_in, ag_a_out) if part == 0 else (ag_b_in, ag_b_out)
                nc.gpsimd.collective_compute(
                    "AllToAll",
                    mybir.AluOpType.bypass,
                    replica_groups=[[0, 1, 2, 3, 4, 5, 6, 7]],
                    ins=[agi[qh][:].opt()],
                    outs=[ago[qh][:].opt()],
                )

            out_r = out.rearrange("(o p) t -> p o t", p=128)

            def proj_dma(qh):
                # at_sb channel slots: 0-3 <- OnA part (heads 3g, 3g+1 of the 4
                # groups), 4-5 <- OnB part (heads 3g+2); wp rows are permuted
                # on the host to match this arrival order.
                at_sb = wpool.tile(
                    [128, 2, DCH, 128], bfl, tag="at", bufs=2, name="at_sb"
                )
                for b in range(2):
                    nc.sync.dma_start(
                        at_sb[:, b, 0:4, :],
                        ag_a_out[qh][b * 512 : (b + 1) * 512, :].rearrange(
                            "(o p) t -> p o t", p=128
                        ),
                    )
                    nc.sync.dma_start(
                        at_sb[:, b, 4:6, :],
                        ag_b_out[qh][b * 256 : (b + 1) * 256, :].rearrange(
                            "(o p) t -> p o t", p=128
                        ),
                    )
                return at_sb

            def proj_mtile(qh, at_sb, m):
                y_ps = sps.tile([128, NH], fp32, tag="s", name="y_ps")[:, :256]
                for kk in range(DCH):
                    nc.tensor.matmul(
                        y_ps[:].rearrange("p (b t) -> p b t", b=2),
                        lhsT=wp_sb[:, kk, m * 128 : (m + 1) * 128],
                        rhs=at_sb[:, :, kk, :],
                        start=(kk == 0),
                        stop=(kk == DCH - 1),
                    )
                y_sb = wpool.tile([128, 256], fp32, tag="y", bufs=4, name="y_sb")
                nc.vector.tensor_scalar_add(y_sb[:], y_ps[:], bp_sb[:, m : m + 1])
                nc.sync.dma_start(out_r[:, m, qh * 256 : (qh + 1) * 256], y_sb[:])

            def proj_pass(qh, at_sb):
                for m in range(DCH):
                    proj_mtile(qh, at_sb, m)

            def warm_fill(n):
                """Junk matmuls that keep the PE HAM-warm while waiting."""
                for _ in range(n):
                    w_ps = sps.tile([128, NH], fp32, tag="s", name="w_ps")[:, :256]
                    nc.tensor.matmul(
                        w_ps[:], lhsT=wp_sb[:, 0, 0:128], rhs=wp_sb[:, 1, 0:256],
                        start=True, stop=True,
                    )

            # ---------------- schedule ----------------
            # Minimal bootstrap so the first exp lands as early as possible:
            # keys/queries for the first chunks only; everything else becomes
            # fill-in work inside the scalar-engine-bound attention units.
            nq = (NK + 511) // 512  # K token-quarters (3 for NK=1152)
            qk_pass("k", 0, 0)
            qk_pass("q", 0, 0)
            qk_pass("q", 0, 1)
            for c in range(min(4, nkch)):
                v_pass(c)

            vs = [lambda c=c: v_pass(c) for c in range(4, nkch)]
            # pair(0) fill-ins with deadlines: v(c) at slot <= c, k0 quarter q
            # by chunk 4q, q2 (qs2) before h2_unit(0).
            ex_p0 = [lambda: qk_pass("q", 1, 0), lambda: qk_pass("q", 1, 1)]
            ex_p0 += vs[:1]
            ex_p0 += [lambda: qk_pass("k", 0, 1)]
            ex_p0 += vs[1:2]
            ex_p0 += [lambda q=q: qk_pass("k", 0, q) for q in range(2, nq)]
            ex_p0 += vs[2:]
            ex_h0 = [lambda q=q: qk_pass("k", 1, q) for q in range(2, nq)]
            ex_h0 += [lambda: qk_pass("q", 0, 2), lambda: qk_pass("q", 0, 3)]
            ex_p1 = [lambda: qk_pass("q", 1, 2), lambda: qk_pass("q", 1, 3)]

            # ---- half 0 (tokens 0-1023). The OnA bounce fires as soon as the
            # pair unit's normalize lands; the collective triggers right after
            # h2's (so every core arrives at the A2A barrier early).
            pair_unit(0, extras=ex_p0[:nkch])
            for f in ex_p0[nkch:]:
                f()
            bounce_a(0)
            a2a(0, 0)
            qk_pass("k", 1, 0)
            qk_pass("k", 1, 1)
            h2_unit(0, extras=ex_h0[:nkch])
            for f in ex_h0[nkch:]:
                f()
            bounce_b(0)
            a2a(0, 1)

            # ---- half 1 (tokens 1024-2047); the half-0 collectives hide
            # under pair(1) and proj#0 runs as fill-in inside h2(1).
            pair_unit(1, extras=ex_p1)
            at0 = proj_dma(0)
            bounce_a(1)
            a2a(1, 0)
            ex_h1 = [None] * max(0, nkch - DCH) + [
                lambda m=m: proj_mtile(0, at0, m) for m in range(DCH)
            ]
            h2_unit(1, extras=ex_h1[:nkch])
            for f in ex_h1[nkch:]:
                if f is not None:
                    f()
            bounce_b(1)
            a2a(1, 1)
            warm_fill(60)
            at1 = proj_dma(1)
            proj_pass(1, at1)

            ops_cm.__exit__(None, None, None)
            sps_cm.__exit__(None, None, None)

    nc.finalize()
    return nc


def _prep(x, mask, w_qkv, w_proj, b_proj):
    """Host-side compaction: per-batch token permutation (kept keys first) and
    per-core input shards."""
    perms, counts = [], []
    for b in range(B):
        perm = np.argsort(1 - mask[b], kind="stable")
        perms.append(perm)
        counts.append(int(mask[b].sum()))
    nkch = max(1, int(np.ceil(max(counts) / 128)))
    NK = nkch * 128

    xTs = []
    mfs = []
    for b in range(B):
        xp = np.ascontiguousarray(x[b][perms[b]].T).astype(bf)
        xTs.append(xp)
        m = np.zeros(NK, dtype=np.float32)
        m[: counts[b]] = 1.0
        mfs.append(np.ascontiguousarray(m.reshape(nkch, 128).T))

    bp_t = np.ascontiguousarray(b_proj.astype(np.float32).reshape(DCH, 128).T)
    # wp rows permuted to the split-A2A arrival order: slots 0-3 carry heads
    # (3g, 3g+1) of group g, slots 4-5 carry heads 3g+2.
    perm_rows = np.empty(C, dtype=np.int64)
    for o in range(4):
        for p in range(128):
            perm_rows[o * 128 + p] = (3 * o + p // 64) * 64 + (p % 64)
    for idx in range(256):
        perm_rows[512 + idx] = (3 * (idx // 64) + 2) * 64 + (idx % 64)
    wp_t = np.ascontiguousarray(w_proj[perm_rows]).astype(bf)

    in_maps = []
    for c in range(NCORES):
        b, g = c // GPB, c % GPB
        heads = [3 * g, 3 * g + 1, 3 * g + 2]
        q_cols = [h * HD + d for h in (heads[0], heads[1], heads[2], heads[2]) for d in range(HD)]
        k_cols = [C + h * HD + d for h in (heads[0], heads[1], heads[2], heads[2]) for d in range(HD)]
        v_cols = [2 * C + h * HD + d for h in heads for d in range(HD)]
        in_maps.append(
            {
                "xT": xTs[b],
                "wq": np.ascontiguousarray(w_qkv[:, q_cols]).astype(bf),
                "wk": np.ascontiguousarray(w_qkv[:, k_cols]).astype(bf),
                "wv": np.ascontiguousarray(w_qkv[:, v_cols]).astype(bf),
                "wp": wp_t,
                "bp": bp_t,
                "mf": mfs[b],
            }
        )
    return in_maps, perms, nkch


def kernel(x, mask, w_qkv, w_proj, b_proj, _trace=False):
    from concourse.bass_utils import run_bass_kernel_spmd

    x = np.asarray(x, dtype=np.float32)
    mask = np.asarray(mask)
    w_qkv = np.asarray(w_qkv, dtype=np.float32)
    w_proj = np.asarray(w_proj, dtype=np.float32)
    b_proj = np.asarray(b_proj, dtype=np.float32)
    in_maps, perms, nkch = _prep(x, mask, w_qkv, w_proj, b_proj)
    if ("nc", nkch) not in _cache:
        _cache[("nc", nkch)] = _build(nkch)
    nc = _cache[("nc", nkch)]
    res = run_bass_kernel_spmd(nc, in_maps, core_ids=list(range(NCORES)), trace=_trace)
    y = np.empty((B, N, C), dtype=np.float32)
    for c in range(NCORES):
        o = np.asarray(res.results[c]["out"])
        for qh in range(2):
            base = qh * NH + c * 128
            for b in range(B):
                y[b, perms[b][base : base + 128]] = o[
                    :, qh * 256 + b * 128 : qh * 256 + (b + 1) * 128
                ].T
    if _trace:
        _cache["last_exec_time_ns"] = res.exec_time_ns
        _cache["last_profile"] = res.profile_json
    return y
